# revision 3
# baseline (speedup 1.0000x reference)
"""Multi-head attention (B=4, N=2048, C=256, H=8, D=32, fp32) on 8 trn2
NeuronCores — v2: two-engine softmax evacuation.

Sharding: data-parallel over batch x query-halves (same as v1, no
collectives). Core c: batch c//2, query rows half=c%2.

Phase B structure (per qt in 2, per head-quad g in 2, per key chunk mc
in 16):
 - stage-1: 4 row-tiled matmuls (K=32 strips) compute the quad's score
   chunks S^T[128 keys, 512 q] into two 2-bank PSUM pair-tiles
   (heads 4g,4g+1 -> tile A; 4g+2,4g+3 -> tile B).
 - The PSUM->SBUF exp evacuation — the throughput bottleneck: both
   PSUM-reading engines run concurrently on different tiles: ScalarE
   does exact exp (ACTIVATE, FD 1024) while VectorE does Schraudolph
   int16 bit-trick exp (bits(bf16) = round(A*x+B) => exp(x) +-3.3%)
   fused into its tensor_scalar copy.
 - stage-2: column-tiled M=33 matmuls (positions (0,0) and (0,64)) run
   pairs of heads concurrently, accumulating z~aug into a 2-bank quad
   tile; the vaug ones-column makes row 32/96 the softmax denominator.
 - per quad: denominators -> DMA-packed [64, 32] -> DVE reciprocal ->
   sel matmul broadcasts to a [128, 512] scale field -> division fused
   into the z~ evacuation (tensor_tensor multiply).
PSUM budget: 3 x 2-bank score tiles + 1 x 2-bank zts + out-proj/szp
rotating through the score pool = 8 banks exactly.
"""

import numpy as np

import concourse.bass as bass
import concourse.mybir as mybir
import concourse.tile as tile
from concourse import bass_utils

B, N, C, H, D = 4, 2048, 256, 8, 32
SCALE = 1.0 / C**0.5
NCORES = 8
NQ = N // 2
QT = NQ // 512
MC = N // 128
F32 = mybir.dt.float32
F32R = mybir.dt.float32r
BF16 = mybir.dt.bfloat16
I16 = mybir.dt.int16
EXP = mybir.ActivationFunctionType.Exp
IDENT = mybir.ActivationFunctionType.Identity
MULT = mybir.AluOpType.mult
ADD = mybir.AluOpType.add

# Schraudolph int16/bf16 exp: bits = round(A16*x + B16); value ~ exp(x)
A16 = (1 << 7) / np.log(2.0)
B16 = 127 * (1 << 7) - 5.5  # shift centers the sawtooth error
# Engine assignment per (mc): 'SD' = tile A on ScalarE, tile B on DVE.
# 'SS' sends both halves to ScalarE (exact exp), rebalancing load.
EVAC_PAT = ["SS" if mc in (3, 8, 13) else "SD" for mc in range(MC)]

_MAXW = 1


def legalize_waits(nc):
    n = 0
    for f in nc.m.functions:
        for bb in f.blocks:
            new = []
            for ins in bb.instructions:
                si = ins.sync_info
                waits = list(si.on_wait) if si and si.on_wait else []
                if len(waits) > _MAXW:
                    si.on_wait = waits[:_MAXW]
                    extra = waits[_MAXW:]
                    for i in range(0, len(extra), _MAXW):
                        n += 1
                        nop = mybir.InstNoOp(name="lw-nop-%d" % n, ins=[], outs=[])
                        nop.engine = ins.engine
                        nop.sync_info = mybir.SyncInfo(
                            on_wait=extra[i : i + _MAXW], on_update=[]
                        )
                        new.append(nop)
                new.append(ins)
            bb.instructions = new


def build_nc(debug=False):
    nc = bass.Bass()

    xT = nc.dram_tensor("xT", (C, N), BF16, kind="ExternalInput")
    wqkvT = nc.dram_tensor("wqkvT", (C, 3 * C), BF16, kind="ExternalInput")
    woutT = nc.dram_tensor("woutT", (C, C), F32R, kind="ExternalInput")
    bqkv_pf = nc.dram_tensor("bqkv_pf", (128, 6), F32, kind="ExternalInput")
    bv_row = nc.dram_tensor("bv_row", (1, C), BF16, kind="ExternalInput")
    bout_pf = nc.dram_tensor("bout_pf", (128, 2), F32, kind="ExternalInput")
    ones_row = nc.dram_tensor("ones_row", (1, 128), BF16, kind="ExternalInput")
    sel4 = nc.dram_tensor("sel4", (4, 128), F32R, kind="ExternalInput")
    ones16 = nc.dram_tensor("ones16", (128, 128), BF16, kind="ExternalInput")
    yT = nc.dram_tensor("yT", (C, NQ), F32, kind="ExternalOutput")

    with tile.TileContext(nc) as tc:
        const = tc.alloc_tile_pool(name="const", bufs=1)

        # ---- load inputs -------------------------------------------------
        xT_sb = const.tile([128, 2, N], BF16, tag="xT")
        for tk in range(2):
            tsl = slice(tk * (N // 2), (tk + 1) * (N // 2))
            nc.sync.dma_start(
                out=xT_sb[:, :, tsl],
                in_=xT.rearrange("(co p) n -> p co n", p=128)[:, :, tsl],
            )
        wqkvT_sb = const.tile([128, 2, 3 * C], BF16, tag="wqkvT")
        nc.sync.dma_start(
            out=wqkvT_sb, in_=wqkvT.rearrange("(co p) o -> p co o", p=128)
        )
        woutT_sb = const.tile([128, 2, C], F32R, tag="woutT")
        nc.sync.dma_start(
            out=woutT_sb, in_=woutT.rearrange("(co p) o -> p co o", p=128)
        )
        bqkv_sb = const.tile([128, 6], F32, tag="bqkv")
        nc.sync.dma_start(out=bqkv_sb, in_=bqkv_pf[:, :])
        bv_sb = const.tile([1, C], BF16, tag="bv")
        nc.sync.dma_start(out=bv_sb, in_=bv_row[:, :])
        bout_sb = const.tile([128, 2], F32, tag="bout")
        nc.sync.dma_start(out=bout_sb, in_=bout_pf[:, :])
        ones_sb = const.tile([1, 128], BF16, tag="ones")
        nc.sync.dma_start(out=ones_sb, in_=ones_row[:, :])
        sel_sb = const.tile([4, 128], F32R, tag="sel")
        nc.sync.dma_start(out=sel_sb, in_=sel4[:, :])

        qT_sb = const.tile([128, 2, NQ], BF16, tag="qT")
        kT_sb = const.tile([128, 2, N], BF16, tag="kT")
        vaug_sb = const.tile([128, MC, H, 33], BF16, tag="vaug")
        nc.sync.dma_start(
            out=vaug_sb[:, :, :, 32],
            in_=ones16[:, 0 : MC * H].rearrange("p (a b) -> p a b", a=MC),
        )
        zT_sb = const.tile([128, 2, NQ], F32R, tag="zT")
        outT_sb = const.tile([128, 2, NQ], F32, tag="outT")

        # ---- phase A: QKV projections -----------------------------------
        psA = tc.alloc_tile_pool(name="psA", bufs=4, space="PSUM")
        # Q^T [256, NQ]; K^T [256, N] — evac on ScalarE (Identity + bias)
        for feat, nts, dst, boff in (
            (0, QT, qT_sb, 0),
            (256, N // 512, kT_sb, 2),
        ):
            for oc in range(2):
                for nt in range(nts):
                    ps = psA.tile([128, 512], F32, tag="qk")
                    for cc in range(2):
                        nc.tensor.matmul(
                            ps,
                            lhsT=wqkvT_sb[
                                :, cc, feat + oc * 128 : feat + (oc + 1) * 128
                            ],
                            rhs=xT_sb[:, cc, nt * 512 : (nt + 1) * 512],
                            start=(cc == 0),
                            stop=(cc == 1),
                        )
                    if True:
                        nc.scalar.activation(
                            out=dst[:, oc, nt * 512 : (nt + 1) * 512],
                            in_=ps,
                            func=IDENT,
                            bias=bqkv_sb[:, boff + oc : boff + oc + 1],
                            scale=1.0,
                        )
                    else:
                        nc.vector.tensor_scalar_add(
                            out=dst[:, oc, nt * 512 : (nt + 1) * 512],
                            in0=ps,
                            scalar1=bqkv_sb[:, boff + oc : boff + oc + 1],
                        )
        # V natural [N, 256] + bias via ones-matmul; evac on DVE
        for mc in range(MC):
            ps = psA.tile([128, C], F32, tag="v")
            for cc in range(2):
                nc.tensor.matmul(
                    ps,
                    lhsT=xT_sb[:, cc, mc * 128 : (mc + 1) * 128],
                    rhs=wqkvT_sb[:, cc, 512:768],
                    start=(cc == 0),
                    stop=False,
                )
            nc.tensor.matmul(
                ps,
                lhsT=ones_sb[0:1, 0:128],
                rhs=bv_sb[0:1, :],
                start=False,
                stop=True,
            )
            ps_v = ps.rearrange("m (h d) -> m h d", h=H)
            nc.vector.tensor_copy(out=vaug_sb[:, mc, :, 0:32], in_=ps_v)
        psA.release()

        # ---- phase B: attention ----------------------------------------
        with tc.tile_pool(name="psB", bufs=1, space="PSUM") as psB, tc.tile_pool(
            name="esb", bufs=12
        ) as esb, tc.tile_pool(name="small", bufs=3) as small, tc.tile_pool(
            name="dsc", bufs=2, space="DRAM"
        ) as dsc:
            for qt in range(QT):
                qsl = slice(qt * 512, (qt + 1) * 512)
                for g in range(2):
                    # z~aug accumulator: bank b holds head 4g+b at psum
                    # partitions [0:33] and head 4g+2+b at [64:97]
                    zts = psB.tile(
                        [128, 2, 512], F32, tag="zt", bufs=2,
                        name="zt%d%d" % (qt, g),
                    )

                    def emit_stage2(mc, eA, eB, g=g, zts=zts):
                        # col-group order (0, 64, 0, 64) so concurrent pairs
                        # land on distinct PE column groups back-to-back
                        for k, (e, ch) in ((0, (eA, 0)), (2, (eB, 0)),
                                           (1, (eA, 1)), (3, (eB, 1))):
                            off = 64 * (k // 2)
                            nc.tensor.matmul(
                                zts[off : off + 33, k % 2, :],
                                lhsT=vaug_sb[:, mc, 4 * g + k, :],
                                rhs=e[:, ch, :],
                                start=(mc == 0),
                                stop=(mc == MC - 1),
                                tile_position=(0, off),
                            )

                    prev = None
                    for mc in range(MC):
                        stA = psB.tile(
                            [128, 2, 512], F32, tag="st", bufs=2,
                            name="stA%d%d%d" % (qt, g, mc),
                        )
                        stB = psB.tile(
                            [128, 2, 512], F32, tag="st", bufs=2,
                            name="stB%d%d%d" % (qt, g, mc),
                        )
                        for j in range(4):
                            st = stA if j < 2 else stB
                            nc.tensor.matmul(
                                st[:, j % 2, :],
                                lhsT=kT_sb[
                                    j * 32 : (j + 1) * 32,
                                    g,
                                    mc * 128 : (mc + 1) * 128,
                                ],
                                rhs=qT_sb[j * 32 : (j + 1) * 32, g, qsl],
                                start=True,
                                stop=True,
                                tile_position=(j * 32, 0),
                            )
                        eA = esb.tile(
                            [128, 2, 512], BF16, tag="E", name="eA%d%d%d" % (qt, g, mc)
                        )
                        eB = esb.tile(
                            [128, 2, 512], BF16, tag="E", name="eB%d%d%d" % (qt, g, mc)
                        )
                        pat = EVAC_PAT[mc]
                        for st, e, eng in ((stA, eA, pat[0]), (stB, eB, pat[1])):
                            if eng == "S":
                                nc.scalar.activation(
                                    out=e, in_=st, func=EXP, scale=SCALE
                                )
                            else:
                                with nc.allow_low_precision(reason="schraudolph"):
                                    nc.vector.tensor_scalar(
                                        out=e.bitcast(I16),
                                        in0=st,
                                        scalar1=float(A16 * SCALE),
                                        scalar2=float(B16),
                                        op0=MULT,
                                        op1=ADD,
                                    )
                        # stage-2 emitted one mc behind (software pipeline)
                        # so its four matmuls issue as one contiguous burst.
                        # tile A holds heads 4g (ch 0), 4g+1 (ch 1); tile B
                        # heads 4g+2, 4g+3. Head 4g+k -> bank k%2, off 64*(k//2)
                        if prev is not None:
                            emit_stage2(*prev)
                        prev = (mc, eA, eB)

                    emit_stage2(*prev)

                    # ---- quad tail: den -> recp -> szp -> divide+evac ----
                    # den rows: partition 32 holds heads (4g, 4g+1),
                    # partition 96 holds (4g+2, 4g+3)
                    den_lo = small.tile([1, 2, 512], F32, tag="denl")
                    den_hi = small.tile([1, 2, 512], F32, tag="denh")
                    nc.vector.tensor_copy(out=den_lo, in_=zts[32:33, :, :])
                    nc.scalar.activation(
                        out=den_hi, in_=zts[96:97, :, :], func=IDENT, scale=1.0
                    )
                    # pack via DRAM roundtrip (partition-restructuring APs
                    # are only safe on the DRAM side of a DMA):
                    # den_pk partition 16k+i = den of head 4g+k (k=2r+b),
                    # query n = 32i + j
                    den_dr = dsc.tile([2, 2, 512], F32, tag="ddr")
                    nc.sync.dma_start(out=den_dr[0:1], in_=den_lo)
                    nc.sync.dma_start(out=den_dr[1:2], in_=den_hi)
                    den_pk = small.tile([64, 32], F32, tag="dpk")
                    nc.sync.dma_start(
                        out=den_pk,
                        in_=den_dr.rearrange("r b (i j) -> (r b i) j", j=32),
                    )
                    recp_pk = small.tile([64, 32], F32R, tag="rpk")
                    with nc.allow_low_precision(reason="fp32r denominators"):
                        nc.vector.reciprocal(out=recp_pk, in_=den_pk)
                    recp_dr = dsc.tile([64, 32], F32R, tag="rdr")
                    nc.sync.dma_start(out=recp_dr, in_=recp_pk)
                    recp_q = small.tile([4, 512], F32R, tag="rq")
                    nc.sync.dma_start(
                        out=recp_q,
                        in_=recp_dr.rearrange("(k i) j -> k (i j)", i=16),
                    )
                    # szp[32k+d, n] = recp of head 4g+k
                    sz = psB.tile([128, 2, 512], F32, tag="st", bufs=2,
                                  name="sz%d%d" % (qt, g))
                    nc.tensor.matmul(
                        sz[:, 0, :], lhsT=sel_sb, rhs=recp_q, start=True, stop=True
                    )
                    szp_sb = small.tile([128, 512], F32, tag="szp")
                    nc.vector.tensor_copy(out=szp_sb, in_=sz[:, 0, :])
                    # fused divide + evacuate (head 4g+k at bank k%2,
                    # partition offset 64*(k//2))
                    for k in range(4):
                        off = 64 * (k // 2)
                        with nc.allow_low_precision(reason="fp32r z"):
                            nc.vector.tensor_tensor(
                                out=zT_sb[32 * k : 32 * k + 32, g, qsl],
                                in0=zts[off : off + 32, k % 2, :],
                                in1=szp_sb[32 * k : 32 * k + 32, :],
                                op=MULT,
                            )

                # ---- out-projection ------------------------------------
                for fc in range(2):
                    op = psB.tile([128, 2, 512], F32, tag="st", bufs=2,
                                  name="op%d%d" % (qt, fc))
                    for dc in range(2):
                        nc.tensor.matmul(
                            op[:, 0, :],
                            lhsT=woutT_sb[:, dc, fc * 128 : (fc + 1) * 128],
                            rhs=zT_sb[:, dc, qsl],
                            start=(dc == 0),
                            stop=(dc == 1),
                        )
                    nc.scalar.activation(
                        out=outT_sb[:, fc, qsl],
                        in_=op[:, 0, :],
                        func=IDENT,
                        bias=bout_sb[:, fc : fc + 1],
                        scale=1.0,
                    )
                nc.sync.dma_start(
                    out=yT.rearrange("(co p) n -> p co n", p=128)[:, :, qsl],
                    in_=outT_sb[:, :, qsl],
                )
                if debug and qt == 0:
                    dbg_zts = const.tile([128, 2, 512], F32, tag="dbgzts")
                    nc.vector.tensor_copy(out=dbg_zts, in_=zts)
                    for name, t in [
                        ("dbg_zts", dbg_zts),
                        ("dbg_denpk", den_pk),
                        ("dbg_recppk", recp_pk),
                        ("dbg_rrow", recp_q),
                        ("dbg_szp", szp_sb),
                    ]:
                        shp = [t.shape[0], int(np.prod(t.shape[1:]))]
                        dt_ = nc.dram_tensor(name, shp, t.dtype, kind="ExternalOutput")
                        nc.sync.dma_start(
                            out=dt_[:, :], in_=t[:].rearrange("p ... -> p (...)")
                        )

        if debug:
            for name, t in [
                ("dbg_qT", qT_sb),
                ("dbg_kT", kT_sb),
                ("dbg_vaug", vaug_sb),
                ("dbg_zT", zT_sb),
            ]:
                shp = [128, int(np.prod(t.shape[1:]))]
                dt_ = nc.dram_tensor(name, shp, t.dtype, kind="ExternalOutput")
                flat = t if len(t.shape) == 2 else (
                    t.rearrange("p a c -> p (a c)") if len(t.shape) == 3
                    else t.rearrange("p a h c -> p (a h c)")
                )
                nc.sync.dma_start(out=dt_[:, :], in_=flat)
        const.release()
    legalize_waits(nc)
    return nc


def make_in_maps(x, w_qkv, b_qkv, w_out, b_out):
    import ml_dtypes

    BF = ml_dtypes.bfloat16
    x = np.ascontiguousarray(x, dtype=np.float32)
    wqkvT = np.ascontiguousarray(np.asarray(w_qkv, np.float32).T.astype(BF))
    woutT = np.ascontiguousarray(np.asarray(w_out, np.float32).T)
    b_qkv = np.asarray(b_qkv, np.float32)
    b_out = np.asarray(b_out, np.float32)
    bqkv_pf = np.ascontiguousarray(b_qkv.reshape(6, 128).T)
    bv_row = np.ascontiguousarray(b_qkv[512:].reshape(1, C).astype(BF))
    bout_pf = np.ascontiguousarray(b_out.reshape(2, 128).T)
    ones_row = np.ones((1, 128), BF)
    sel4 = np.zeros((4, 128), np.float32)
    for k in range(4):
        sel4[k, 32 * k : 32 * k + 32] = 1.0
    ones16 = np.ones((128, 128), BF)

    in_maps = []
    for c in range(NCORES):
        b, half = c // 2, c % 2
        xTb = x[b].T
        if half:
            xTb = np.concatenate([xTb[:, NQ:], xTb[:, :NQ]], axis=1)
        in_maps.append(
            {
                "xT": np.ascontiguousarray(xTb.astype(BF)),
                "wqkvT": wqkvT,
                "woutT": woutT,
                "bqkv_pf": bqkv_pf,
                "bv_row": bv_row,
                "bout_pf": bout_pf,
                "ones_row": ones_row,
                "sel4": sel4,
                "ones16": ones16,
            }
        )
    return in_maps


def assemble(results):
    out = np.empty((B, N, C), dtype=np.float32)
    for c in range(NCORES):
        b, half = c // 2, c % 2
        out[b, half * NQ : (half + 1) * NQ, :] = results[c]["yT"].T
    return out


_NC_CACHE = {}


def kernel(x, w_qkv, b_qkv, w_out, b_out):
    if "nc" not in _NC_CACHE:
        _NC_CACHE["nc"] = build_nc()
    nc = _NC_CACHE["nc"]
    in_maps = make_in_maps(x, w_qkv, b_qkv, w_out, b_out)
    # The first execution after a NEFF load has produced corrupted outputs
    # intermittently (device-state initialization issue); the second
    # execution has been reliable across every observed instance. Run the
    # kernel twice and return the second result (~35us extra device time).
    bass_utils.run_bass_kernel_spmd(nc, in_maps, core_ids=list(range(NCORES)))
    res = bass_utils.run_bass_kernel_spmd(nc, in_maps, core_ids=list(range(NCORES)))
    return assemble(res.results)



# revision 5
# speedup vs baseline: 1.3614x; 1.3614x over previous
"""Multi-head attention (B=4, N=2048, C=256, H=8, D=32, fp32) on 8 trn2
NeuronCores — v4: pipelined softmax tails.

Sharding: data-parallel over batch x query-halves (no collectives).
Core c: batch c//2, query rows half=c%2.

Phase B structure (per qt in 2, per head-quad g in 2, per key chunk mc
in 16):
 - stage-1: 4 row-tiled matmuls (K=32 strips) compute the quad's score
   chunks S^T[128 keys, 512 q] into two 2-bank PSUM pair-tiles
   (heads 4g,4g+1 -> tile A; 4g+2,4g+3 -> tile B).
 - The PSUM->SBUF exp evacuation — the throughput bottleneck: both
   PSUM-reading engines run concurrently on different tiles: ScalarE
   does exact exp (ACTIVATE, FD 1024) while VectorE does Schraudolph
   int16 bit-trick exp (bits(bf16) = round(A*x+B) => exp(x) +-3.3%)
   fused into its tensor_scalar copy.
 - stage-2: column-tiled M=33 matmuls (positions (0,0) and (0,64)) run
   pairs of heads concurrently, accumulating z~aug into a 2-bank quad
   tile; the vaug ones-column makes row 32/96 the softmax denominator.
 - quad tail (den rows -> DMA pack [64,32] -> DVE reciprocal -> DMA
   replication-broadcast to a [128, 512] scale field -> divide fused
   into the z~ evacuation) is SOFTWARE-PIPELINED: emitted one step per
   mc into the NEXT quad's loop, so the strict-FIFO engine queues never
   head-of-line block on the tail's DMA latencies, the PE never idles
   long enough for HAM to re-throttle the clock, and the double-
   buffered zts lets the next quad accumulate immediately.
PSUM budget: 2 x (2-bank score pair-tile) + 2 x (2-bank zts) = 8 banks;
out-proj briefly rotates through the score tag.
"""

import numpy as np

import concourse.bass as bass
import concourse.mybir as mybir
import concourse.tile as tile
from concourse import bass_utils

B, N, C, H, D = 4, 2048, 256, 8, 32
SCALE = 1.0 / C**0.5
NCORES = 8
NQ = N // 2
QT = NQ // 512
MC = N // 128
F32 = mybir.dt.float32
F32R = mybir.dt.float32r
BF16 = mybir.dt.bfloat16
I16 = mybir.dt.int16
EXP = mybir.ActivationFunctionType.Exp
IDENT = mybir.ActivationFunctionType.Identity
MULT = mybir.AluOpType.mult
ADD = mybir.AluOpType.add

# Schraudolph int16/bf16 exp: bits = round(A16*x + B16); value ~ exp(x)
A16 = (1 << 7) / np.log(2.0)
B16 = 127 * (1 << 7) - 5.5  # shift centers the sawtooth error
# Engine assignment per (mc): 'SD' = tile A on ScalarE, tile B on DVE.
# 'SS' sends both halves to ScalarE (exact exp), rebalancing load.
EVAC_PAT = ["SS" if mc in (3, 8, 13) else "SD" for mc in range(MC)]

_MAXW = 1


def legalize_waits(nc):
    n = 0
    for f in nc.m.functions:
        for bb in f.blocks:
            new = []
            for ins in bb.instructions:
                si = ins.sync_info
                waits = list(si.on_wait) if si and si.on_wait else []
                if len(waits) > _MAXW:
                    si.on_wait = waits[:_MAXW]
                    extra = waits[_MAXW:]
                    for i in range(0, len(extra), _MAXW):
                        n += 1
                        nop = mybir.InstNoOp(name="lw-nop-%d" % n, ins=[], outs=[])
                        nop.engine = ins.engine
                        nop.sync_info = mybir.SyncInfo(
                            on_wait=extra[i : i + _MAXW], on_update=[]
                        )
                        new.append(nop)
                new.append(ins)
            bb.instructions = new


def build_nc(debug=False):
    nc = bass.Bass()

    xT = nc.dram_tensor("xT", (C, N), BF16, kind="ExternalInput")
    wqkvT = nc.dram_tensor("wqkvT", (C, 3 * C), BF16, kind="ExternalInput")
    woutT = nc.dram_tensor("woutT", (C, C), F32R, kind="ExternalInput")
    bqkv_pf = nc.dram_tensor("bqkv_pf", (128, 6), F32, kind="ExternalInput")
    bv_row = nc.dram_tensor("bv_row", (1, C), BF16, kind="ExternalInput")
    bout_pf = nc.dram_tensor("bout_pf", (128, 2), F32, kind="ExternalInput")
    ones_row = nc.dram_tensor("ones_row", (1, 128), BF16, kind="ExternalInput")
    ones16 = nc.dram_tensor("ones16", (128, 128), BF16, kind="ExternalInput")
    yT = nc.dram_tensor("yT", (C, NQ), F32, kind="ExternalOutput")

    with tile.TileContext(nc) as tc:
        const = tc.alloc_tile_pool(name="const", bufs=1)

        # ---- load inputs -------------------------------------------------
        ones_sb = const.tile([1, 128], BF16, tag="ones")
        nc.sync.dma_start(out=ones_sb, in_=ones_row[:, :])
        # warm the ScalarE exp table set while input DMAs stream
        scr_sb = const.tile([1, 128], BF16, tag="scr")
        nc.scalar.activation(out=scr_sb, in_=ones_sb, func=EXP, scale=1.0)

        xT_sb = const.tile([128, 2, N], BF16, tag="xT")
        for tk in range(2):
            tsl = slice(tk * (N // 2), (tk + 1) * (N // 2))
            nc.sync.dma_start(
                out=xT_sb[:, :, tsl],
                in_=xT.rearrange("(co p) n -> p co n", p=128)[:, :, tsl],
            )
        wqkvT_sb = const.tile([128, 2, 3 * C], BF16, tag="wqkvT")
        nc.sync.dma_start(
            out=wqkvT_sb, in_=wqkvT.rearrange("(co p) o -> p co o", p=128)
        )
        woutT_sb = const.tile([128, 2, C], F32R, tag="woutT")
        nc.sync.dma_start(
            out=woutT_sb, in_=woutT.rearrange("(co p) o -> p co o", p=128)
        )
        bqkv_sb = const.tile([128, 6], F32, tag="bqkv")
        nc.sync.dma_start(out=bqkv_sb, in_=bqkv_pf[:, :])
        bv_sb = const.tile([1, C], BF16, tag="bv")
        nc.sync.dma_start(out=bv_sb, in_=bv_row[:, :])
        bout_sb = const.tile([128, 2], F32, tag="bout")
        nc.sync.dma_start(out=bout_sb, in_=bout_pf[:, :])

        qT_sb = const.tile([128, 2, NQ], BF16, tag="qT")
        kT_sb = const.tile([128, 2, N], BF16, tag="kT")
        vaug_sb = const.tile([128, MC, H, 33], BF16, tag="vaug")
        nc.sync.dma_start(
            out=vaug_sb[:, :, :, 32],
            in_=ones16[:, 0 : MC * H].rearrange("p (a b) -> p a b", a=MC),
        )
        zT_sb = const.tile([128, 2, NQ], F32R, tag="zT")
        outT_sb = const.tile([128, 2, NQ], F32, tag="outT")

        # ---- phase A: QKV projections -----------------------------------
        psA = tc.alloc_tile_pool(name="psA", bufs=4, space="PSUM")
        # Q^T [256, NQ]; K^T [256, N] — evac on ScalarE (Identity + bias)
        for feat, nts, dst, boff in (
            (0, QT, qT_sb, 0),
            (256, N // 512, kT_sb, 2),
        ):
            for oc in range(2):
                for nt in range(nts):
                    ps = psA.tile([128, 512], F32, tag="qk")
                    for cc in range(2):
                        nc.tensor.matmul(
                            ps,
                            lhsT=wqkvT_sb[
                                :, cc, feat + oc * 128 : feat + (oc + 1) * 128
                            ],
                            rhs=xT_sb[:, cc, nt * 512 : (nt + 1) * 512],
                            start=(cc == 0),
                            stop=(cc == 1),
                        )
                    nc.scalar.activation(
                        out=dst[:, oc, nt * 512 : (nt + 1) * 512],
                        in_=ps,
                        func=IDENT,
                        bias=bqkv_sb[:, boff + oc : boff + oc + 1],
                        scale=1.0,
                    )
        # V natural [N, 256] + bias via ones-matmul; evac on DVE
        for mc in range(MC):
            ps = psA.tile([128, C], F32, tag="v")
            for cc in range(2):
                nc.tensor.matmul(
                    ps,
                    lhsT=xT_sb[:, cc, mc * 128 : (mc + 1) * 128],
                    rhs=wqkvT_sb[:, cc, 512:768],
                    start=(cc == 0),
                    stop=False,
                )
            nc.tensor.matmul(
                ps,
                lhsT=ones_sb[0:1, 0:128],
                rhs=bv_sb[0:1, :],
                start=False,
                stop=True,
            )
            ps_v = ps.rearrange("m (h d) -> m h d", h=H)
            nc.vector.tensor_copy(out=vaug_sb[:, mc, :, 0:32], in_=ps_v)
        psA.release()

        # ---- phase B: attention ----------------------------------------
        with tc.tile_pool(name="psB", bufs=1, space="PSUM") as psB, tc.tile_pool(
            name="esb", bufs=12
        ) as esb, tc.tile_pool(name="small", bufs=2) as small, tc.tile_pool(
            name="dsc", bufs=2, space="DRAM"
        ) as dsc:

            def make_tail(qt, g, zts):
                """Tail of quad (qt, g): denominators -> reciprocal ->
                broadcast scale field -> fused divide+evac of z~.
                Returned as a list of step-closures to be emitted one
                per mc inside the NEXT quad's loop (pipelined past the
                DMA latencies)."""
                qsl = slice(qt * 512, (qt + 1) * 512)
                uid = "%d%d" % (qt, g)
                den_lo = small.tile([1, 2, 512], F32, tag="denl",
                                    name="denl" + uid)
                den_hi = small.tile([1, 2, 512], F32, tag="denh",
                                    name="denh" + uid)
                den_dr = dsc.tile([2, 2, 512], F32, tag="ddr", name="ddr" + uid)
                den_pk = small.tile([64, 32], F32, tag="dpk", name="dpk" + uid)
                recp_pk = small.tile([64, 32], F32R, tag="rpk",
                                     name="rpk" + uid)
                recp_dr = dsc.tile([64, 32], F32R, tag="rdr", name="rdr" + uid)
                szp_sb = small.tile([128, 512], F32R, tag="szp",
                                    name="szp" + uid)

                # den rows: partition 32 holds heads (4g, 4g+1),
                # partition 96 holds (4g+2, 4g+3); per head the den of
                # query n=512*qt+q sits at free offset (head%2)*512+q.
                def s_den():
                    nc.vector.tensor_copy(out=den_lo, in_=zts[32:33, :, :])
                    nc.scalar.activation(
                        out=den_hi, in_=zts[96:97, :, :], func=IDENT, scale=1.0
                    )
                    nc.sync.dma_start(out=den_dr[0:1], in_=den_lo)
                    nc.sync.dma_start(out=den_dr[1:2], in_=den_hi)

                # pack via DRAM roundtrip (partition-restructuring APs are
                # only safe on the DRAM side of a DMA): den_pk partition
                # 16k+i = den of head 4g+k (k=2r+b), query n = 32i + j
                def s_pack():
                    nc.sync.dma_start(
                        out=den_pk,
                        in_=den_dr.rearrange("r b (i j) -> (r b i) j", j=32),
                    )

                def s_recp():
                    with nc.allow_low_precision(reason="fp32r denominators"):
                        nc.vector.reciprocal(out=recp_pk, in_=den_pk)
                    nc.sync.dma_start(out=recp_dr, in_=recp_pk)

                # szp[32k+d, q] = recp of head 4g+k: replication-broadcast
                # DMA (0-stride middle dim on the DRAM side)
                def s_szp():
                    nc.sync.dma_start(
                        out=szp_sb[:, :],
                        in_=recp_dr.rearrange("(k i) j -> k (i j)", i=16)[
                            :, None, :
                        ].to_broadcast([4, 32, 512]),
                    )

                def make_div(k):
                    def s_div():
                        off = 64 * (k // 2)
                        with nc.allow_low_precision(reason="fp32r z"):
                            nc.vector.tensor_tensor(
                                out=zT_sb[32 * k : 32 * k + 32, g, qsl],
                                in0=zts[off : off + 32, k % 2, :],
                                in1=szp_sb[32 * k : 32 * k + 32, :],
                                op=MULT,
                            )

                    return s_div

                return [s_den, s_pack, s_recp, s_szp] + [
                    make_div(k) for k in range(4)
                ]

            def make_outproj(qt):
                """Out-projection for query block qt (needs zT of both
                head-quads, i.e. the tails of (qt, 0) and (qt, 1))."""
                qsl = slice(qt * 512, (qt + 1) * 512)

                def make_fc(fc):
                    def s_fc():
                        op = psB.tile([128, 2, 512], F32, tag="st", bufs=2,
                                      name="op%d%d" % (qt, fc))
                        for dc in range(2):
                            nc.tensor.matmul(
                                op[:, 0, :],
                                lhsT=woutT_sb[:, dc, fc * 128 : (fc + 1) * 128],
                                rhs=zT_sb[:, dc, qsl],
                                start=(dc == 0),
                                stop=(dc == 1),
                            )
                        nc.scalar.activation(
                            out=outT_sb[:, fc, qsl],
                            in_=op[:, 0, :],
                            func=IDENT,
                            bias=bout_sb[:, fc : fc + 1],
                            scale=1.0,
                        )

                    return s_fc

                def s_store():
                    nc.sync.dma_start(
                        out=yT.rearrange("(co p) n -> p co n", p=128)[:, :, qsl],
                        in_=outT_sb[:, :, qsl],
                    )

                return [make_fc(0), make_fc(1), s_store]

            # pending pipeline steps from the previous quad, emitted one
            # per mc into the current quad's loop (tail first, then the
            # previous qt's out-projection once both its tails are in).
            pending = []

            def run_pending():
                if pending:
                    pending.pop(0)()

            quads = [(qt, g) for qt in range(QT) for g in range(2)]
            for qi, (qt, g) in enumerate(quads):
                qsl = slice(qt * 512, (qt + 1) * 512)
                # z~aug accumulator: bank b holds head 4g+b at psum
                # partitions [0:33] and head 4g+2+b at [64:97]
                zts = psB.tile(
                    [128, 2, 512], F32, tag="zt", bufs=2,
                    name="zt%d%d" % (qt, g),
                )

                def emit_stage2(mc, eA, eB, g=g, zts=zts):
                    # col-group order (0, 64, 0, 64) so concurrent pairs
                    # land on distinct PE column groups back-to-back
                    for k, (e, ch) in ((0, (eA, 0)), (2, (eB, 0)),
                                       (1, (eA, 1)), (3, (eB, 1))):
                        off = 64 * (k // 2)
                        nc.tensor.matmul(
                            zts[off : off + 33, k % 2, :],
                            lhsT=vaug_sb[:, mc, 4 * g + k, :],
                            rhs=e[:, ch, :],
                            start=(mc == 0),
                            stop=(mc == MC - 1),
                            tile_position=(0, off),
                        )

                prev = None
                for mc in range(MC):
                    stA = psB.tile(
                        [128, 2, 512], F32, tag="st", bufs=2,
                        name="stA%d%d%d" % (qt, g, mc),
                    )
                    stB = psB.tile(
                        [128, 2, 512], F32, tag="st", bufs=2,
                        name="stB%d%d%d" % (qt, g, mc),
                    )
                    for j in range(4):
                        st = stA if j < 2 else stB
                        nc.tensor.matmul(
                            st[:, j % 2, :],
                            lhsT=kT_sb[
                                j * 32 : (j + 1) * 32,
                                g,
                                mc * 128 : (mc + 1) * 128,
                            ],
                            rhs=qT_sb[j * 32 : (j + 1) * 32, g, qsl],
                            start=True,
                            stop=True,
                            tile_position=(j * 32, 0),
                        )
                    eA = esb.tile(
                        [128, 2, 512], BF16, tag="E", name="eA%d%d%d" % (qt, g, mc)
                    )
                    eB = esb.tile(
                        [128, 2, 512], BF16, tag="E", name="eB%d%d%d" % (qt, g, mc)
                    )
                    pat = EVAC_PAT[mc]
                    for st, e, eng in ((stA, eA, pat[0]), (stB, eB, pat[1])):
                        if eng == "S":
                            nc.scalar.activation(
                                out=e, in_=st, func=EXP, scale=SCALE
                            )
                        else:
                            with nc.allow_low_precision(reason="schraudolph"):
                                nc.vector.tensor_scalar(
                                    out=e.bitcast(I16),
                                    in0=st,
                                    scalar1=float(A16 * SCALE),
                                    scalar2=float(B16),
                                    op0=MULT,
                                    op1=ADD,
                                )
                    # previous quad's pipelined tail / out-proj steps
                    run_pending()
                    # stage-2 emitted one mc behind (software pipeline)
                    # so its four matmuls issue as one contiguous burst.
                    # tile A holds heads 4g (ch 0), 4g+1 (ch 1); tile B
                    # heads 4g+2, 4g+3. Head 4g+k -> bank k%2, off 64*(k//2)
                    if prev is not None:
                        emit_stage2(*prev)
                    prev = (mc, eA, eB)

                emit_stage2(*prev)

                # queue this quad's tail; after the g=1 tail, also queue
                # the out-projection of this qt.
                pending.extend(make_tail(qt, g, zts))
                if g == 1:
                    pending.extend(make_outproj(qt))

            # drain remaining pipeline steps (last quad's tail + final
            # out-projection)
            while pending:
                run_pending()

        const.release()
    legalize_waits(nc)
    return nc


def make_in_maps(x, w_qkv, b_qkv, w_out, b_out):
    import ml_dtypes

    BF = ml_dtypes.bfloat16
    x = np.ascontiguousarray(x, dtype=np.float32)
    wqkvT = np.ascontiguousarray(np.asarray(w_qkv, np.float32).T.astype(BF))
    woutT = np.ascontiguousarray(np.asarray(w_out, np.float32).T)
    b_qkv = np.asarray(b_qkv, np.float32)
    b_out = np.asarray(b_out, np.float32)
    bqkv_pf = np.ascontiguousarray(b_qkv.reshape(6, 128).T)
    bv_row = np.ascontiguousarray(b_qkv[512:].reshape(1, C).astype(BF))
    bout_pf = np.ascontiguousarray(b_out.reshape(2, 128).T)
    ones_row = np.ones((1, 128), BF)
    ones16 = np.ones((128, 128), BF)

    in_maps = []
    for c in range(NCORES):
        b, half = c // 2, c % 2
        xTb = x[b].T
        if half:
            xTb = np.concatenate([xTb[:, NQ:], xTb[:, :NQ]], axis=1)
        in_maps.append(
            {
                "xT": np.ascontiguousarray(xTb.astype(BF)),
                "wqkvT": wqkvT,
                "woutT": woutT,
                "bqkv_pf": bqkv_pf,
                "bv_row": bv_row,
                "bout_pf": bout_pf,
                "ones_row": ones_row,
                "ones16": ones16,
            }
        )
    return in_maps


def assemble(results):
    out = np.empty((B, N, C), dtype=np.float32)
    for c in range(NCORES):
        b, half = c // 2, c % 2
        out[b, half * NQ : (half + 1) * NQ, :] = results[c]["yT"].T
    return out


_NC_CACHE = {}


def kernel(x, w_qkv, b_qkv, w_out, b_out):
    if "nc" not in _NC_CACHE:
        _NC_CACHE["nc"] = build_nc()
    nc = _NC_CACHE["nc"]
    in_maps = make_in_maps(x, w_qkv, b_qkv, w_out, b_out)
    # The first execution after a NEFF load has produced corrupted outputs
    # intermittently (device-state initialization issue); the second
    # execution has been reliable across every observed instance. Run the
    # kernel twice and return the second result (~35us extra device time).
    bass_utils.run_bass_kernel_spmd(nc, in_maps, core_ids=list(range(NCORES)))
    res = bass_utils.run_bass_kernel_spmd(nc, in_maps, core_ids=list(range(NCORES)))
    return assemble(res.results)


# revision 7
# speedup vs baseline: 1.4945x; 1.0977x over previous
"""Multi-head attention (B=4, N=2048, C=256, H=8, D=32, fp32) on 8 trn2
NeuronCores — v5: head-pair (duo) tiling + pipelined softmax tails.

Sharding: data-parallel over batch x query-halves (no collectives).
Core c: batch c//2, query rows half=c%2.

Phase B processes 8 "duos" (qt in 2 x head-pair d in 4), 16 key chunks
(mc) each:
 - stage-1: 2 row-tiled matmuls (K=32 strips at row positions of the
   two heads) compute S^T[128 keys, 512 q] for both heads into ONE
   2-bank PSUM pair-tile (tag st, bufs=3 -> 6 banks: the PE can run
   several chunks ahead of the evacuation, keeping its instruction
   queue dense so the HAM clock-gate stays at full rate).
 - evacuation: ONE instruction per chunk [128, 2x512] alternating
   between ScalarE (exact exp ACTIVATE) and VectorE (Schraudolph int16
   bit-trick exp fused into tensor_scalar) per EVAC_PAT for load
   balance; both engines run concurrently on different chunks.
 - stage-2: two column-tiled M=33 matmuls (positions (0,0) and (0,64))
   accumulate z~aug for both heads into a SINGLE-bank zts (tag zt,
   bufs=2 -> 2 banks): rows 0:33 head A, 64:97 head B; the vaug ones
   column makes rows 32/96 the softmax denominators.
 - duo tail (den rows -> DMA pack [32,32] -> DVE reciprocal -> DMA
   replication-broadcast to a [64, 512] scale field -> divide fused
   into the z~ evacuation) is SOFTWARE-PIPELINED one step per mc into
   the NEXT duo's loop so the strict-FIFO engine queues never stall on
   DMA latency, and the double-buffered zts lets the next duo
   accumulate immediately.
PSUM budget: 3 x (2-bank score pair) + 2 x (1-bank zts) = 8 banks;
out-proj briefly rotates through the score tag.
"""

import numpy as np

import concourse.bass as bass
import concourse.mybir as mybir
import concourse.tile as tile
from concourse import bass_utils

B, N, C, H, D = 4, 2048, 256, 8, 32
SCALE = 1.0 / C**0.5
NCORES = 8
NQ = N // 2
QT = NQ // 512
MC = N // 128
ND = 4  # head-pair duos per qt
F32 = mybir.dt.float32
F32R = mybir.dt.float32r
BF16 = mybir.dt.bfloat16
I16 = mybir.dt.int16
EXP = mybir.ActivationFunctionType.Exp
IDENT = mybir.ActivationFunctionType.Identity
MULT = mybir.AluOpType.mult
ADD = mybir.AluOpType.add

# Schraudolph int16/bf16 exp: bits = round(A16*x + B16); value ~ exp(x)
A16 = (1 << 7) / np.log(2.0)
B16 = 127 * (1 << 7) - 5.5  # shift centers the sawtooth error
# Evac engine per mc: 'S' ScalarE exact exp, 'D' VectorE Schraudolph.
# 9 S / 7 D balances ScalarE against VectorE + its tail work (den copy,
# reciprocal, divides); DVE evacs sit on even-ish slots away from the
# early-mc tail steps.
EVAC_PAT = ["D" if mc in (3, 5, 7, 9, 11, 13, 15) else "S" for mc in range(MC)]

_MAXW = 1


def legalize_waits(nc):
    n = 0
    for f in nc.m.functions:
        for bb in f.blocks:
            new = []
            for ins in bb.instructions:
                si = ins.sync_info
                waits = list(si.on_wait) if si and si.on_wait else []
                if len(waits) > _MAXW:
                    si.on_wait = waits[:_MAXW]
                    extra = waits[_MAXW:]
                    for i in range(0, len(extra), _MAXW):
                        n += 1
                        nop = mybir.InstNoOp(name="lw-nop-%d" % n, ins=[], outs=[])
                        nop.engine = ins.engine
                        nop.sync_info = mybir.SyncInfo(
                            on_wait=extra[i : i + _MAXW], on_update=[]
                        )
                        new.append(nop)
                new.append(ins)
            bb.instructions = new


def build_nc(debug=False):
    nc = bass.Bass()

    xT = nc.dram_tensor("xT", (C, N), BF16, kind="ExternalInput")
    wqkvT = nc.dram_tensor("wqkvT", (C, 3 * C), BF16, kind="ExternalInput")
    woutT = nc.dram_tensor("woutT", (C, C), F32R, kind="ExternalInput")
    bqkv_pf = nc.dram_tensor("bqkv_pf", (128, 6), F32, kind="ExternalInput")
    bv_row = nc.dram_tensor("bv_row", (1, C), F32, kind="ExternalInput")
    bout_pf = nc.dram_tensor("bout_pf", (128, 2), F32, kind="ExternalInput")
    ones_row = nc.dram_tensor("ones_row", (1, 128), BF16, kind="ExternalInput")
    ones16 = nc.dram_tensor("ones16", (128, 128), BF16, kind="ExternalInput")
    yT = nc.dram_tensor("yT", (C, NQ), F32, kind="ExternalOutput")

    with tile.TileContext(nc) as tc:
        const = tc.alloc_tile_pool(name="const", bufs=1)

        # ---- load inputs -------------------------------------------------
        ones_sb = const.tile([1, 128], BF16, tag="ones")
        nc.sync.dma_start(out=ones_sb, in_=ones_row[:, :])
        # warm the ScalarE exp table set while input DMAs stream
        scr_sb = const.tile([1, 128], BF16, tag="scr")
        nc.scalar.activation(out=scr_sb, in_=ones_sb, func=EXP, scale=1.0)

        xT_sb = const.tile([128, 2, N], BF16, tag="xT")
        for tk in range(2):
            tsl = slice(tk * (N // 2), (tk + 1) * (N // 2))
            nc.sync.dma_start(
                out=xT_sb[:, :, tsl],
                in_=xT.rearrange("(co p) n -> p co n", p=128)[:, :, tsl],
            )
        wqkvT_sb = const.tile([128, 2, 3 * C], BF16, tag="wqkvT")
        nc.sync.dma_start(
            out=wqkvT_sb, in_=wqkvT.rearrange("(co p) o -> p co o", p=128)
        )
        woutT_sb = const.tile([128, 2, C], F32R, tag="woutT")
        nc.sync.dma_start(
            out=woutT_sb, in_=woutT.rearrange("(co p) o -> p co o", p=128)
        )
        bqkv_sb = const.tile([128, 6], F32, tag="bqkv")
        nc.sync.dma_start(out=bqkv_sb, in_=bqkv_pf[:, :])
        # V bias replicated to all partitions (0-stride broadcast read)
        bvb_sb = const.tile([128, C], F32, tag="bvb")
        nc.sync.dma_start(out=bvb_sb, in_=bv_row[:, :].to_broadcast([128, C]))
        bout_sb = const.tile([128, 2], F32, tag="bout")
        nc.sync.dma_start(out=bout_sb, in_=bout_pf[:, :])

        qT_sb = const.tile([128, 2, NQ], BF16, tag="qT")
        kT_sb = const.tile([128, 2, N], BF16, tag="kT")
        vaug_sb = const.tile([128, MC, H, 33], BF16, tag="vaug")
        nc.sync.dma_start(
            out=vaug_sb[:, :, :, 32],
            in_=ones16[:, 0 : MC * H].rearrange("p (a b) -> p a b", a=MC),
        )
        zT_sb = const.tile([128, 2, NQ], F32R, tag="zT")
        outT_sb = const.tile([128, 2, NQ], F32, tag="outT")

        # ---- phase A: QKV projections -----------------------------------
        psA = tc.alloc_tile_pool(name="psA", bufs=4, space="PSUM")
        # Q^T [256, NQ]; K^T [256, N] — evac on ScalarE (Identity + bias)
        for feat, nts, dst, boff in (
            (0, QT, qT_sb, 0),
            (256, N // 512, kT_sb, 2),
        ):
            for oc in range(2):
                for nt in range(nts):
                    ps = psA.tile([128, 512], F32, tag="qk")
                    for cc in range(2):
                        nc.tensor.matmul(
                            ps,
                            lhsT=wqkvT_sb[
                                :, cc, feat + oc * 128 : feat + (oc + 1) * 128
                            ],
                            rhs=xT_sb[:, cc, nt * 512 : (nt + 1) * 512],
                            start=(cc == 0),
                            stop=(cc == 1),
                        )
                    nc.scalar.activation(
                        out=dst[:, oc, nt * 512 : (nt + 1) * 512],
                        in_=ps,
                        func=IDENT,
                        bias=bqkv_sb[:, boff + oc : boff + oc + 1],
                        scale=1.0,
                    )
        # V natural [N, 256]; bias folded into the DVE evacuation
        bvb_r = bvb_sb.rearrange("p (h d) -> p h d", h=H)
        for mc in range(MC):
            ps = psA.tile([128, C], F32, tag="v")
            for cc in range(2):
                nc.tensor.matmul(
                    ps,
                    lhsT=xT_sb[:, cc, mc * 128 : (mc + 1) * 128],
                    rhs=wqkvT_sb[:, cc, 512:768],
                    start=(cc == 0),
                    stop=(cc == 1),
                )
            ps_v = ps.rearrange("m (h d) -> m h d", h=H)
            nc.vector.tensor_tensor(
                out=vaug_sb[:, mc, :, 0:32], in0=ps_v, in1=bvb_r, op=ADD
            )
        psA.release()

        # ---- phase B: attention ----------------------------------------
        with tc.tile_pool(name="psB", bufs=1, space="PSUM") as psB, tc.tile_pool(
            name="esb", bufs=12
        ) as esb, tc.tile_pool(name="small", bufs=2) as small, tc.tile_pool(
            name="dsc", bufs=2, space="DRAM"
        ) as dsc:

            def make_tail(qt, d, zts):
                """Tail of duo (qt, d): denominators -> reciprocal ->
                broadcast scale field -> fused divide+evac of z~, as a
                list of step-closures emitted one per mc inside the
                NEXT duo's loop."""
                qsl = slice(qt * 512, (qt + 1) * 512)
                uid = "%d%d" % (qt, d)
                den_lo = small.tile([1, 512], F32, tag="denl", name="dl" + uid)
                den_hi = small.tile([1, 512], F32, tag="denh", name="dh" + uid)
                den_dr = dsc.tile([2, 512], F32, tag="ddr", name="ddr" + uid)
                den_pk = small.tile([32, 32], F32, tag="dpk", name="dpk" + uid)
                recp_pk = small.tile([32, 32], F32R, tag="rpk", name="rpk" + uid)
                recp_dr = dsc.tile([32, 32], F32R, tag="rdr", name="rdr" + uid)
                szp_sb = small.tile([64, 512], F32R, tag="szp", name="szp" + uid)

                # den rows: partition 32 = head 2d, partition 96 = head 2d+1
                def s_den():
                    nc.vector.tensor_copy(out=den_lo, in_=zts[32:33, :])
                    nc.scalar.activation(
                        out=den_hi, in_=zts[96:97, :], func=IDENT, scale=1.0
                    )
                    nc.sync.dma_start(out=den_dr[0:1], in_=den_lo)
                    nc.sync.dma_start(out=den_dr[1:2], in_=den_hi)

                # pack via DRAM roundtrip (partition-restructuring APs are
                # only safe on the DRAM side of a DMA): den_pk partition
                # 16k+i = den of head 2d+k, query n = 32i + j
                def s_pack():
                    nc.sync.dma_start(
                        out=den_pk,
                        in_=den_dr.rearrange("r (i j) -> (r i) j", j=32),
                    )

                def s_recp():
                    with nc.allow_low_precision(reason="fp32r denominators"):
                        nc.vector.reciprocal(out=recp_pk, in_=den_pk)
                    nc.sync.dma_start(out=recp_dr, in_=recp_pk)

                # szp[32k+dd, q] = recp of head 2d+k: replication-broadcast
                # DMA (0-stride middle dim on the DRAM side)
                def s_szp():
                    nc.sync.dma_start(
                        out=szp_sb[:, :],
                        in_=recp_dr.rearrange("(k i) j -> k (i j)", i=16)[
                            :, None, :
                        ].to_broadcast([2, 32, 512]),
                    )

                def make_div(k):
                    def s_div():
                        h = 2 * d + k
                        with nc.allow_low_precision(reason="fp32r z"):
                            nc.vector.tensor_tensor(
                                out=zT_sb[
                                    (h % 4) * 32 : (h % 4) * 32 + 32, h // 4, qsl
                                ],
                                in0=zts[64 * k : 64 * k + 32, :],
                                in1=szp_sb[32 * k : 32 * k + 32, :],
                                op=MULT,
                            )

                    return s_div

                return [s_den, s_pack, s_recp, s_szp, make_div(0), make_div(1)]

            def make_outproj(qt):
                """Out-projection for query block qt (needs the tails of
                all four duos (qt, 0..3))."""
                qsl = slice(qt * 512, (qt + 1) * 512)

                def make_fc(fc):
                    def s_fc():
                        op = psB.tile([128, 2, 512], F32, tag="st", bufs=3,
                                      name="op%d%d" % (qt, fc))
                        for dc in range(2):
                            nc.tensor.matmul(
                                op[:, 0, :],
                                lhsT=woutT_sb[:, dc, fc * 128 : (fc + 1) * 128],
                                rhs=zT_sb[:, dc, qsl],
                                start=(dc == 0),
                                stop=(dc == 1),
                            )
                        nc.scalar.activation(
                            out=outT_sb[:, fc, qsl],
                            in_=op[:, 0, :],
                            func=IDENT,
                            bias=bout_sb[:, fc : fc + 1],
                            scale=1.0,
                        )

                    return s_fc

                def s_store():
                    nc.sync.dma_start(
                        out=yT.rearrange("(co p) n -> p co n", p=128)[:, :, qsl],
                        in_=outT_sb[:, :, qsl],
                    )

                return [make_fc(0), make_fc(1), s_store]

            # pending pipeline steps from the previous duo, emitted one
            # per mc into the current duo's loop.
            pending = []

            def run_pending():
                if pending:
                    pending.pop(0)()

            duos = [(qt, d) for qt in range(QT) for d in range(ND)]
            for qt, d in duos:
                qsl = slice(qt * 512, (qt + 1) * 512)
                h0 = 2 * d  # first head of the duo
                oc = h0 // 4  # C-half holding this duo's features
                r0 = (h0 % 4) * 32  # kT/qT partition strip of head h0
                # z~aug accumulator (1 bank): rows 0:33 head h0 (den at
                # 32), rows 64:97 head h0+1 (den at 96)
                zts = psB.tile(
                    [128, 512], F32, tag="zt", bufs=2, name="zt%d%d" % (qt, d)
                )

                def emit_stage2(mc, e, d=d, zts=zts):
                    for k in range(2):
                        nc.tensor.matmul(
                            zts[64 * k : 64 * k + 33, :],
                            lhsT=vaug_sb[:, mc, 2 * d + k, :],
                            rhs=e[:, k, :],
                            start=(mc == 0),
                            stop=(mc == MC - 1),
                            tile_position=(0, 64 * k),
                        )

                prev = None
                for mc in range(MC):
                    st = psB.tile(
                        [128, 2, 512], F32, tag="st", bufs=3,
                        name="st%d%d%d" % (qt, d, mc),
                    )
                    for k in range(2):
                        nc.tensor.matmul(
                            st[:, k, :],
                            lhsT=kT_sb[
                                r0 + 32 * k : r0 + 32 * (k + 1),
                                oc,
                                mc * 128 : (mc + 1) * 128,
                            ],
                            rhs=qT_sb[r0 + 32 * k : r0 + 32 * (k + 1), oc, qsl],
                            start=True,
                            stop=True,
                            tile_position=(r0 + 32 * k, 0),
                        )
                    e = esb.tile(
                        [128, 2, 512], BF16, tag="E", name="e%d%d%d" % (qt, d, mc)
                    )
                    if EVAC_PAT[mc] == "S":
                        nc.scalar.activation(out=e, in_=st, func=EXP, scale=SCALE)
                    else:
                        with nc.allow_low_precision(reason="schraudolph"):
                            nc.vector.tensor_scalar(
                                out=e.bitcast(I16),
                                in0=st,
                                scalar1=float(A16 * SCALE),
                                scalar2=float(B16),
                                op0=MULT,
                                op1=ADD,
                            )
                    # previous duo's pipelined tail / out-proj steps
                    run_pending()
                    # stage-2 one mc behind (software pipeline)
                    if prev is not None:
                        emit_stage2(*prev)
                    prev = (mc, e)

                emit_stage2(*prev)

                # queue this duo's tail; after the d=3 tail, also queue
                # the out-projection of this qt.
                pending.extend(make_tail(qt, d, zts))
                if d == ND - 1:
                    pending.extend(make_outproj(qt))

            # drain remaining pipeline steps (last duo's tail + final
            # out-projection)
            while pending:
                run_pending()

        const.release()
    legalize_waits(nc)
    return nc


def make_in_maps(x, w_qkv, b_qkv, w_out, b_out):
    import ml_dtypes

    BF = ml_dtypes.bfloat16
    x = np.ascontiguousarray(x, dtype=np.float32)
    wqkvT = np.ascontiguousarray(np.asarray(w_qkv, np.float32).T.astype(BF))
    woutT = np.ascontiguousarray(np.asarray(w_out, np.float32).T)
    b_qkv = np.asarray(b_qkv, np.float32)
    b_out = np.asarray(b_out, np.float32)
    bqkv_pf = np.ascontiguousarray(b_qkv.reshape(6, 128).T)
    bv_row = np.ascontiguousarray(b_qkv[512:].reshape(1, C))
    bout_pf = np.ascontiguousarray(b_out.reshape(2, 128).T)
    ones_row = np.ones((1, 128), BF)
    ones16 = np.ones((128, 128), BF)

    in_maps = []
    for c in range(NCORES):
        b, half = c // 2, c % 2
        xTb = x[b].T
        if half:
            xTb = np.concatenate([xTb[:, NQ:], xTb[:, :NQ]], axis=1)
        in_maps.append(
            {
                "xT": np.ascontiguousarray(xTb.astype(BF)),
                "wqkvT": wqkvT,
                "woutT": woutT,
                "bqkv_pf": bqkv_pf,
                "bv_row": bv_row,
                "bout_pf": bout_pf,
                "ones_row": ones_row,
                "ones16": ones16,
            }
        )
    return in_maps


def assemble(results):
    out = np.empty((B, N, C), dtype=np.float32)
    for c in range(NCORES):
        b, half = c // 2, c % 2
        out[b, half * NQ : (half + 1) * NQ, :] = results[c]["yT"].T
    return out


_NC_CACHE = {}


def kernel(x, w_qkv, b_qkv, w_out, b_out):
    if "nc" not in _NC_CACHE:
        _NC_CACHE["nc"] = build_nc()
    nc = _NC_CACHE["nc"]
    in_maps = make_in_maps(x, w_qkv, b_qkv, w_out, b_out)
    # The first execution after a NEFF load has produced corrupted outputs
    # intermittently (device-state initialization issue); the second
    # execution has been reliable across every observed instance. Run the
    # kernel twice and return the second result (~35us extra device time).
    bass_utils.run_bass_kernel_spmd(nc, in_maps, core_ids=list(range(NCORES)))
    res = bass_utils.run_bass_kernel_spmd(nc, in_maps, core_ids=list(range(NCORES)))
    return assemble(res.results)
